# revision 1
# baseline (speedup 1.0000x reference)
"""Trainium2 Bass kernel for nn_CriticReadoutModule (twin-critic GNN readout).

Computes, for each sample b:
  x_o   = concat(obs[b,:64], act[b], ctx[b,o])            # [264] per object
  h_o   = relu(x_o @ W1c + b1c)                           # [256] per critic c
  p     = sum_o relu(h_o @ W2c + b2c)                     # [192]
  q_c   = relu(p @ R1c + rb1c) @ R2c + rb2c               # scalar
Returns (q1, q2), each [B, 1] float32.

Strategy: pure data parallelism over batch across 8 cores. On-chip layout is
feature-major (features on SBUF partitions, batch moving on the free dim);
the host pre-transposes/splits x into K-chunks of 128/128/8 rows. All matmul
operands are fp16 (PSUM accumulation stays fp32) — fp16 runs at the same PE
rate as bf16/f32r here but with 4x the mantissa, and halves HBM traffic.
The emission is software-pipelined: the PE instruction stream is
... L1(s), L2(s-1), L1(s+1), L2(s), [RHO], ... so the in-order PE queue never
waits on the ACT relu between an object's L1 and its L2. The object-sum pool
uses sum_o relu(y+b) = sum_o max(y,-b) + 10*b, with the 10*b correction
folded into rho's first-layer bias on the host, so the whole L2 epilogue is a
single fused DVE op per object and bank: acc = max(psum, -b2) + acc.

(The local name `bf16` below is historical — it holds mybir.dt.float16.)
"""

import sys
import numpy as np

for _p in ("/opt/trn_rl_repo",):
    if _p not in sys.path:
        sys.path.insert(0, _p)

N_CORES = 8
DIM_BODY = 64
DIM_ACT = 16
DBODY = DIM_BODY + DIM_ACT  # 80
O = 10
DCTX = 184
HID = 256
DPHI = 192
NT = 512  # moving-dim tile (one fp32 PSUM bank)


def build_nc(S, repeat=1):
    """Build + compile the per-core Bass module for a shard of S samples.

    repeat>1 wraps the whole computation in a hardware loop that redoes it
    (on the same data) repeat times — used only for timing measurements.
    """
    import concourse.bacc as bacc
    import concourse.bass as bass
    import concourse.mybir as mybir
    import concourse.tile as tile

    f32 = mybir.dt.float32
    bf16 = mybir.dt.float16
    AF = mybir.ActivationFunctionType
    ALU = mybir.AluOpType

    assert S % NT == 0
    BT = S // NT

    nc = bacc.Bacc("TRN2", target_bir_lowering=False, debug=False)

    # ---- DRAM I/O ----
    # x layout: logical rows 0..79 = body, 80..263 = ctx. Chunked for the PE:
    #   x1 = rows 0..127 (body + ctx[0:48])   [O, 128, S]  (body replicated per object on host)
    #   x2 = rows 128..255 (ctx[48:176])      [O, 128, S]
    #   xlo = rows 256..263 (ctx[176:184]) x4 copies for row-tiled matmuls [O, 32, S]
    x1_d = nc.dram_tensor("x1", [O, 128, S], bf16, kind="ExternalInput")
    x2_d = nc.dram_tensor("x2", [O, 128, S], bf16, kind="ExternalInput")
    xlo_d = nc.dram_tensor("xlo", [O, 32, S], bf16, kind="ExternalInput")
    w1_d = nc.dram_tensor("w1s", [DBODY + DCTX, 2 * HID], bf16, kind="ExternalInput")
    w1lo_d = nc.dram_tensor("w1lo", [32, 2 * HID], bf16, kind="ExternalInput")
    w2a_d = nc.dram_tensor("w2a", [HID, DPHI], bf16, kind="ExternalInput")
    w2b_d = nc.dram_tensor("w2b", [HID, DPHI], bf16, kind="ExternalInput")
    rw1a_d = nc.dram_tensor("rw1a", [DPHI, HID], bf16, kind="ExternalInput")
    rw1b_d = nc.dram_tensor("rw1b", [DPHI, HID], bf16, kind="ExternalInput")
    rw2a_d = nc.dram_tensor("rw2a", [HID, 1], bf16, kind="ExternalInput")
    rw2b_d = nc.dram_tensor("rw2b", [HID, 1], bf16, kind="ExternalInput")
    b1_d = nc.dram_tensor("b1s", [4, 128], f32, kind="ExternalInput")
    nb2_128_d = nc.dram_tensor("nb2_128", [2, 128], f32, kind="ExternalInput")
    nb2_64_d = nc.dram_tensor("nb2_64", [2, 64], f32, kind="ExternalInput")
    rb1_d = nc.dram_tensor("rb1adj", [4, 128], f32, kind="ExternalInput")
    rb2_d = nc.dram_tensor("rb2", [2, 1], f32, kind="ExternalInput")
    q1_d = nc.dram_tensor("q1", [S, 1], f32, kind="ExternalOutput")
    q2_d = nc.dram_tensor("q2", [S, 1], f32, kind="ExternalOutput")

    with tile.TileContext(nc) as tc:
        with (
            tc.tile_pool(name="wp", bufs=1) as wp,
            tc.tile_pool(name="dp", bufs=1) as dp,
            tc.tile_pool(name="pp", bufs=1, space="PSUM") as pp,
        ):
            # ---- static weights / biases (loaded once) ----
            w_a = wp.tile([128, 2 * HID], bf16, name="w_a")
            w_b = wp.tile([128, 2 * HID], bf16, name="w_b")
            nc.sync.dma_start(w_a[:], w1_d[0:128, :])
            nc.sync.dma_start(w_b[:], w1_d[128:256, :])
            # 4 copies of the 8-row tail of W1, at partition offsets 0/32/64/96
            w_lo = wp.tile([104, 2 * HID], bf16, name="w_lo")
            for m in range(4):
                nc.sync.dma_start(w_lo[32 * m : 32 * m + 8, :], w1lo_d[8 * m : 8 * m + 8, :])

            w2 = {}
            for cn, wd in (("a", w2a_d), ("b", w2b_d)):
                for k in range(2):
                    t = wp.tile([128, DPHI], bf16, name=f"w2{cn}k{k}")
                    nc.sync.dma_start(t[:], wd[k * 128 : (k + 1) * 128, :])
                    w2[cn, k] = t
            rw1 = {}
            for cn, wd in (("a", rw1a_d), ("b", rw1b_d)):
                t0 = wp.tile([128, HID], bf16, name=f"rw1{cn}0")
                nc.sync.dma_start(t0[:], wd[0:128, :])
                rw1[cn] = t0
            # 64-row tails of rho_w1 for both critics, stacked so the two
            # K=64 matmuls land in disjoint PE row groups and run concurrently
            rw1t = wp.tile([128, HID], bf16, name="rw1t")
            nc.sync.dma_start(rw1t[0:64, :], rw1a_d[128:DPHI, :])
            nc.sync.dma_start(rw1t[64:128, :], rw1b_d[128:DPHI, :])
            rw2 = {}
            for cn, wd in (("a", rw2a_d), ("b", rw2b_d)):
                t0 = wp.tile([128, 1], bf16, name=f"rw2{cn}0")
                t1 = wp.tile([128, 1], bf16, name=f"rw2{cn}1")
                nc.sync.dma_start(t0[:], wd[0:128, :])
                nc.sync.dma_start(t1[:], wd[128:HID, :])
                rw2[cn] = (t0, t1)

            b1c = []
            for m in range(4):
                t = wp.tile([128, 1], f32, name=f"b1c{m}")
                nc.sync.dma_start(t[:], b1_d[m, :])
                b1c.append(t)
            nb2 = {}
            nb2t = wp.tile([128, 1], f32, name="nb2t")
            for i, cn in enumerate(("a", "b")):
                t0 = wp.tile([128, 1], f32, name=f"nb2{cn}0")
                nc.sync.dma_start(t0[:], nb2_128_d[i, :])
                nc.sync.dma_start(nb2t[64 * i : 64 * i + 64, :], nb2_64_d[i, :])
                nb2[cn] = (t0, nb2t)
            rb1c = []
            for m in range(4):
                t = wp.tile([128, 1], f32, name=f"rb1c{m}")
                nc.sync.dma_start(t[:], rb1_d[m, :])
                rb1c.append(t)
            rb2c = []
            for i in range(2):
                t = wp.tile([1, 1], f32, name=f"rb2c{i}")
                nc.sync.dma_start(t[:], rb2_d[i, :])
                rb2c.append(t)

            # ---- main loop ----
            import contextlib

            rep_ctx = tc.For_i(0, repeat, 1) if repeat > 1 else contextlib.nullcontext()
            with rep_ctx:
                _main_body(
                    nc, tc, dp, pp, BT, f32, bf16, AF, ALU,
                    x1_d, x2_d, xlo_d, q1_d, q2_d,
                    w_a, w_b, w_lo, w2, rw1, rw1t, rw2,
                    b1c, nb2, rb1c, rb2c,
                )

    nc.compile()
    return nc


def _main_body(nc, tc, dp, pp, BT, f32, bf16, AF, ALU, x1_d, x2_d, xlo_d, q1_d, q2_d,
               w_a, w_b, w_lo, w2, rw1, rw1t, rw2, b1c, nb2, rb1c, rb2c):
    """Software-pipelined emission: the PE instruction stream is
    ... L1(s), L2(s-1), L1(s+1), L2(s), [RHO], ... so the in-order PE queue
    never stalls waiting for the ACT relu between L1 and L2 of the same
    object. rho(bt) is delayed two objects past L2(bt, O-1) for the same
    reason (its input is produced by the DVE epilogue)."""

    def emit_l1(bt, o, acc, acc1, bs):
        x1 = dp.tile([128, NT], bf16, name="x1", tag="x1", bufs=4)
        x2 = dp.tile([128, NT], bf16, name="x2", tag="x2", bufs=4)
        xlo = dp.tile([104, NT], bf16, name="xlo", tag="xlo", bufs=4)
        nc.sync.dma_start(x1[:], x1_d[o, :, bs])
        nc.sync.dma_start(x2[:], x2_d[o, :, bs])
        for m in range(4):
            nc.sync.dma_start(xlo[32 * m : 32 * m + 8, :], xlo_d[o, 8 * m : 8 * m + 8, bs])
        # 8 full-K matmuls, then the four 8-row tails back-to-back in
        # disjoint row groups so they overlap on the PE array.
        ph = []
        for m in range(4):
            ms = slice(m * 128, (m + 1) * 128)
            p = pp.tile([128, NT], f32, name="ph", tag="ph", bufs=4)
            nc.tensor.matmul(p[:], w_a[:, ms], x1[:], start=True, stop=False)
            nc.tensor.matmul(p[:], w_b[:, ms], x2[:], start=False, stop=False)
            ph.append(p)
        for m in range(4):
            ms = slice(m * 128, (m + 1) * 128)
            rs = slice(32 * m, 32 * m + 8)
            nc.tensor.matmul(
                ph[m][:], w_lo[rs, ms], xlo[rs, :], start=False, stop=True,
                tile_position=(32 * m, 0),
            )
        h = []
        for m in range(4):
            ht = dp.tile([128, NT], bf16, name="h", tag="h", bufs=12)
            nc.scalar.activation(ht[:], ph[m][:], AF.Relu, bias=b1c[m][:])
            h.append(ht)
        return h

    def emit_l2(bt, o, h, acc, acc1):
        for cc, (cn, h0, h1) in enumerate((("a", h[0], h[1]), ("b", h[2], h[3]))):
            for part in range(2):
                mslice = slice(0, 128) if part == 0 else slice(128, DPHI)
                mp = 128 if part == 0 else DPHI - 128
                ppt = pp.tile([mp, NT], f32, name="pp2", tag=f"pp2_{part}", bufs=2)
                nc.tensor.matmul(ppt[:], w2[cn, 0][:, mslice], h0[:], start=True, stop=False)
                nc.tensor.matmul(ppt[:], w2[cn, 1][:, mslice], h1[:], start=False, stop=True)
                if part == 0:
                    a = acc[cn][:]
                    nb = nb2[cn][0][:]
                else:
                    a = acc1[64 * cc : 64 * cc + 64, :]
                    nb = nb2[cn][1][64 * cc : 64 * cc + 64, :]
                if o == 0:
                    nc.vector.tensor_scalar(a, ppt[:], nb, None, ALU.max)
                else:
                    nc.vector.scalar_tensor_tensor(
                        a, ppt[:], nb, a, op0=ALU.max, op1=ALU.add
                    )

    def emit_rho(acc, acc1, bs):
        psz = {}
        for ci, cn in enumerate(("a", "b")):
            for m in range(2):
                ms = slice(m * 128, (m + 1) * 128)
                p = pp.tile([128, NT], f32, name="psz", tag="ph", bufs=4)
                nc.tensor.matmul(p[:], rw1[cn][:, ms], acc[cn][:], start=True, stop=False)
                psz[cn, m] = p
        for ci, cn in enumerate(("a", "b")):
            rs = slice(64 * ci, 64 * ci + 64)
            for m in range(2):
                ms = slice(m * 128, (m + 1) * 128)
                nc.tensor.matmul(psz[cn, m][:], rw1t[rs, ms], acc1[rs, :], start=False, stop=True)
        for ci, cn in enumerate(("a", "b")):
            zr = []
            for m in range(2):
                zt = dp.tile([128, NT], bf16, name="zr", tag="zr", bufs=4)
                nc.scalar.activation(zt[:], psz[cn, m][:], AF.Relu, bias=rb1c[2 * ci + m][:])
                zr.append(zt)
            psq = pp.tile([1, NT], f32, name="psq", tag="pp2_1", bufs=2)
            nc.tensor.matmul(psq[:], rw2[cn][0][:], zr[0][:], start=True, stop=False)
            nc.tensor.matmul(psq[:], rw2[cn][1][:], zr[1][:], start=False, stop=True)
            qt = dp.tile([1, NT], f32, name="qt", tag="qt", bufs=4)
            nc.scalar.activation(qt[:], psq[:], AF.Identity, bias=rb2c[ci][:])
            qd = q1_d if ci == 0 else q2_d
            nc.sync.dma_start(qd[bs, :], qt[:])

    pend_l2 = None   # (bt, o, h, acc, acc1, bs) awaiting L2 emission
    rho_q = []       # [steps_to_wait, (acc, acc1, bs)]
    for bt in range(BT):
        bs = slice(bt * NT, (bt + 1) * NT)
        acc = {
            "a": dp.tile([128, NT], bf16, name="acA0", tag="acc128", bufs=4),
            "b": dp.tile([128, NT], bf16, name="acB0", tag="acc128", bufs=4),
        }
        acc1 = dp.tile([128, NT], bf16, name="acc1", tag="acc64", bufs=4)
        for o in range(O):
            h = emit_l1(bt, o, acc, acc1, bs)
            if pend_l2 is not None:
                pbt, po, ph_, pacc, pacc1, pbs = pend_l2
                emit_l2(pbt, po, ph_, pacc, pacc1)
                if po == O - 1:
                    rho_q.append([1, (pacc, pacc1, pbs)])
            pend_l2 = (bt, o, h, acc, acc1, bs)
            for item in rho_q:
                item[0] -= 1
            if rho_q and rho_q[0][0] < 0:
                _, args = rho_q.pop(0)
                emit_rho(*args)
    # drain
    pbt, po, ph_, pacc, pacc1, pbs = pend_l2
    emit_l2(pbt, po, ph_, pacc, pacc1)
    rho_q.append([0, (pacc, pacc1, pbs)])
    for _, args in rho_q:
        emit_rho(*args)


def prep_inputs(inputs, S, core):
    """Host-side shard + layout prep for one core. Returns the in_map."""
    import ml_dtypes

    bf = np.float16
    lo, hi = core * S, (core + 1) * S
    obs = inputs["obs"][lo:hi]
    act = inputs["act"][lo:hi]
    ctx = inputs["context_layer"][lo:hi]
    body_t = np.concatenate([obs[:, :DIM_BODY], act], axis=1).T  # [80, S]
    ctx_t = np.transpose(ctx, (1, 2, 0))  # [O, 184, S]
    # x chunks (see build_nc): x1 = body ++ ctx[0:48], x2 = ctx[48:176],
    # xlo = ctx[176:184] replicated 4x for the row-tiled tail matmuls
    x1 = np.concatenate(
        [np.broadcast_to(body_t[None], (O, DBODY, S)), ctx_t[:, 0:48]], axis=1
    )
    x2 = ctx_t[:, 48:176]
    xlo = np.tile(ctx_t[:, 176:184], (1, 4, 1))

    w1s = np.concatenate([inputs["phi_w1a"], inputs["phi_w1b"]], axis=1)
    w1lo = np.tile(w1s[256:264], (4, 1))
    b1s = np.concatenate([inputs["phi_b1a"], inputs["phi_b1b"]]).reshape(4, 128)
    b2a, b2b = inputs["phi_b2a"], inputs["phi_b2b"]
    nb2_128 = np.stack([-b2a[0:128], -b2b[0:128]])
    nb2_64 = np.stack([-b2a[128:DPHI], -b2b[128:DPHI]])
    # rho1 bias adjusted for the +O*b2 correction of the max-based object sum
    rb1a = inputs["rho_b1a"] + O * (b2a @ inputs["rho_w1a"])
    rb1b = inputs["rho_b1b"] + O * (b2b @ inputs["rho_w1b"])
    rb1adj = np.concatenate([rb1a, rb1b]).reshape(4, 128)
    rb2 = np.stack([inputs["rho_b2a"], inputs["rho_b2b"]]).reshape(2, 1)

    f = np.float32
    return {
        "x1": np.ascontiguousarray(x1).astype(bf),
        "x2": np.ascontiguousarray(x2).astype(bf),
        "xlo": np.ascontiguousarray(xlo).astype(bf),
        "w1lo": np.ascontiguousarray(w1lo).astype(bf),
        "w1s": np.ascontiguousarray(w1s).astype(bf),
        "w2a": np.ascontiguousarray(inputs["phi_w2a"]).astype(bf),
        "w2b": np.ascontiguousarray(inputs["phi_w2b"]).astype(bf),
        "rw1a": np.ascontiguousarray(inputs["rho_w1a"]).astype(bf),
        "rw1b": np.ascontiguousarray(inputs["rho_w1b"]).astype(bf),
        "rw2a": np.ascontiguousarray(inputs["rho_w2a"]).astype(bf),
        "rw2b": np.ascontiguousarray(inputs["rho_w2b"]).astype(bf),
        "b1s": np.ascontiguousarray(b1s, f),
        "nb2_128": np.ascontiguousarray(nb2_128, f),
        "nb2_64": np.ascontiguousarray(nb2_64, f),
        "rb1adj": np.ascontiguousarray(rb1adj, f),
        "rb2": np.ascontiguousarray(rb2, f),
    }


_CACHE = {}


def kernel(**inputs):
    from concourse.bass_utils import run_bass_kernel_spmd

    B = inputs["obs"].shape[0]
    assert B % N_CORES == 0
    S = B // N_CORES

    if S not in _CACHE:
        _CACHE[S] = build_nc(S)
    nc = _CACHE[S]

    in_maps = [prep_inputs(inputs, S, c) for c in range(N_CORES)]
    res = run_bass_kernel_spmd(nc, in_maps, list(range(N_CORES)))
    q1 = np.concatenate([res.results[c]["q1"] for c in range(N_CORES)], axis=0)
    q2 = np.concatenate([res.results[c]["q2"] for c in range(N_CORES)], axis=0)
    return (q1.astype(np.float32), q2.astype(np.float32))


if __name__ == "__main__":
    # smoke test with random data
    rng = np.random.default_rng(0)
    B = 32768
    ins = {
        "obs": rng.standard_normal((B, 100), dtype=np.float32),
        "act": rng.standard_normal((B, DIM_ACT), dtype=np.float32),
        "context_layer": rng.standard_normal((B, O, DCTX), dtype=np.float32),
    }
    for n, shp in (
        ("phi_w1a", (264, 256)), ("phi_b1a", (256,)),
        ("phi_w2a", (256, 192)), ("phi_b2a", (192,)),
        ("phi_w1b", (264, 256)), ("phi_b1b", (256,)),
        ("phi_w2b", (256, 192)), ("phi_b2b", (192,)),
        ("rho_w1a", (192, 256)), ("rho_b1a", (256,)),
        ("rho_w2a", (256, 1)), ("rho_b2a", (1,)),
        ("rho_w1b", (192, 256)), ("rho_b1b", (256,)),
        ("rho_w2b", (256, 1)), ("rho_b2b", (1,)),
    ):
        ins[n] = (rng.standard_normal(shp) * 0.05).astype(np.float32)
    q1, q2 = kernel(**ins)
    print(q1.shape, q2.shape, q1[:4, 0], q2[:4, 0])



# revision 2
# speedup vs baseline: 1.0314x; 1.0314x over previous
"""Trainium2 Bass kernel for nn_CriticReadoutModule (twin-critic GNN readout).

v7 = body-split + full-K padding + balanced epilogue.

HW findings driving this design (measured via microbenchmarks + repeat-slope):
- matmul K=128,N=512 costs ~262-287ns; ANY partial-K (8/32/64/92) costs
  ~425-456ns -> every matmul here is padded to K=128 with static zero weight
  rows (and zero/garbage rhs rows that multiply them).
- The body contribution Z = body_act @ W1[:80] + b1 is shared by all 10
  objects: computed once per batch-tile (4 padded matmuls) instead of being
  folded into every object's L1. Per-object L1 contracts only ctx
  (K=184 -> 128 + 56pad128 = 2 matmuls per m-group).
- Per-object relu becomes h = max(psum, -Z) + Z, done as
  ACT copy (psum->fp16) + DVE max (2x) + DVE add (2x); one of the three L2
  epilogue accumulations also routes through an ACT copy so DVE and PE land
  at ~equal per-object time.
Object-sum pool via sum_o relu(y+b2) = sum_o max(y,-b2) + 10*b2 folded into
rho's first-layer bias. Data parallel over batch across 8 cores.
"""

import sys
import numpy as np

for _p in ("/opt/trn_rl_repo",):
    if _p not in sys.path:
        sys.path.insert(0, _p)

N_CORES = 8
DIM_BODY = 64
DIM_ACT = 16
DBODY = DIM_BODY + DIM_ACT  # 80
O = 10
DCTX = 184
HID = 256
DPHI = 192
NT = 512

# weight mega-array column layout (fp16, [128, WCOLS]); all regions K-padded
WC_BODY = 0      # w1s rows 0:80 (rows 80:128 zero)    [128, 512]
WC_C1 = 512      # w1s rows 80:208                      [128, 512]
WC_C2 = 1024     # w1s rows 208:264 (rows 56:128 zero)  [128, 512]
WC_W2K0 = 1536   # packed W2 rows 0:128    [128, 384]
WC_W2K1 = 1920   # packed W2 rows 128:256  [128, 384]
WC_RW1A = 2304   # rho_w1a rows 0:128      [128, 256]
WC_RW1B = 2560   # rho_w1b rows 0:128      [128, 256]
WC_RWTA = 2816   # parts 0:64 = rho_w1a rows 128:192, 64:128 zero  [128, 256]
WC_RWTB = 3072   # parts 0:64 zero, 64:128 = rho_w1b rows 128:192  [128, 256]
WC_RW2 = 3328    # rw2a (2 cols), rw2b (2 cols)
WCOLS = 3332

# bias mega-array column layout (f32, [128, BCOLS])
BC_B1 = 0     # 4 cols: concat(phi_b1a, phi_b1b) 128-chunks (for Z)
BC_NB1 = 4    # 4 cols: negated b1 (for negZ)
BC_NB2 = 8    # 3 cols: -b2 per L2 psum tile (a 0:128, b 0:128, [a 128:192|b 128:192])
BC_RB1 = 11   # 4 cols: rb1adj (a 0:128, a 128:256, b 0:128, b 128:256)
BC_RB2 = 15   # 2 cols: rb2a (part 0), rb2b (part 64)
BCOLS = 17


def build_nc(S, repeat=1, no_xdma=False):
    import concourse.bacc as bacc
    import concourse.mybir as mybir
    import concourse.tile as tile

    f32 = mybir.dt.float32
    f16 = mybir.dt.float16
    AF = mybir.ActivationFunctionType
    ALU = mybir.AluOpType

    assert S % NT == 0
    BT = S // NT

    nc = bacc.Bacc("TRN2", target_bir_lowering=False, debug=False)

    # xo: per object [128, 2NT]: cols 0:NT = ctx rows 0:128; cols NT:2NT =
    # ctx rows 128:184 in partitions 0:56, zeros in 56:128 (host-zeroed).
    xo_d = nc.dram_tensor("xo", [BT, O, 128, 2 * NT], f16, kind="ExternalInput")
    # body: [128, NT] with rows 80:128 host-zeroed.
    xbody_d = nc.dram_tensor("xbody", [BT, 128, NT], f16, kind="ExternalInput")
    wm_d = nc.dram_tensor("wmega", [128, WCOLS], f16, kind="ExternalInput")
    bm_d = nc.dram_tensor("bmega", [128, BCOLS], f32, kind="ExternalInput")
    q_d = nc.dram_tensor("q", [2, S], f32, kind="ExternalOutput")

    with tile.TileContext(nc) as tc:
        with (
            tc.tile_pool(name="wp", bufs=1) as wp,
            tc.tile_pool(name="dp", bufs=1) as dp,
            tc.tile_pool(name="pp", bufs=1, space="PSUM") as pp,
        ):
            wm = wp.tile([128, WCOLS], f16, name="wm")
            bm = wp.tile([128, BCOLS], f32, name="bm")
            nc.sync.dma_start(wm[:], wm_d[:, :])
            nc.sync.dma_start(bm[:], bm_d[:, :])

            import contextlib

            rep_ctx = tc.For_i(0, repeat, 1) if repeat > 1 else contextlib.nullcontext()
            with rep_ctx:
                _main_body(nc, tc, dp, pp, BT, f32, f16, AF, ALU,
                           xo_d, xbody_d, q_d, wm, bm, no_xdma)

    nc.compile()
    return nc


def _main_body(nc, tc, dp, pp, BT, f32, f16, AF, ALU, xo_d, xbody_d, q_d, wm, bm,
               no_xdma=False):

    def emit_body(bt):
        """Z[m] = body@W1[:80] + b1 (fp16), negZ[m] = -(Z[m]); once per tile."""
        xb = dp.tile([128, NT], f16, name="xb", tag="xb", bufs=2)
        if not no_xdma:
            nc.sync.dma_start(xb[:], xbody_d[bt])
        else:
            nc.gpsimd.memset(xb[:], 0.25)
        Z, nZ = [], []
        for m in range(4):
            pz = pp.tile([128, NT], f32, name="pz", tag="ph", bufs=4)
            nc.tensor.matmul(pz[:], wm[:, WC_BODY + m * 128 : WC_BODY + (m + 1) * 128], xb[:], start=True, stop=True)
            zt = dp.tile([128, NT], f16, name="Z", tag="Z", bufs=8)
            nzt = dp.tile([128, NT], f16, name="nZ", tag="nZ", bufs=8)
            nc.scalar.activation(zt[:], pz[:], AF.Identity, bias=bm[:, BC_B1 + m : BC_B1 + m + 1])
            nc.scalar.activation(nzt[:], pz[:], AF.Identity, scale=-1.0, bias=bm[:, BC_NB1 + m : BC_NB1 + m + 1])
            Z.append(zt)
            nZ.append(nzt)
        return Z, nZ

    def emit_l1(bt, o, xo, Z, nZ):
        ph = []
        for m in range(4):
            p = pp.tile([128, NT], f32, name="ph", tag="ph", bufs=4)
            nc.tensor.matmul(p[:], wm[:, WC_C1 + m * 128 : WC_C1 + (m + 1) * 128], xo[:, 0:NT], start=True, stop=False)
            nc.tensor.matmul(p[:], wm[:, WC_C2 + m * 128 : WC_C2 + (m + 1) * 128], xo[:, NT : 2 * NT], start=False, stop=True)
            ph.append(p)
        h = []
        for m in range(4):
            # h = relu(psum + Z) = max(psum, -Z) + Z
            st = dp.tile([128, NT], f16, name="s", tag="s", bufs=8)
            nc.scalar.activation(st[:], ph[m][:], AF.Identity)
            tt = dp.tile([128, NT], f16, name="t", tag="t", bufs=8)
            nc.vector.tensor_tensor(tt[:], st[:], nZ[m][:], ALU.max)
            ht = dp.tile([128, NT], f16, name="h", tag="h", bufs=12)
            nc.vector.tensor_tensor(ht[:], tt[:], Z[m][:], ALU.add)
            h.append(ht)
        return h

    def emit_l2(o, h, acc):
        tiles = []
        for c in range(3):
            ppt = pp.tile([128, NT], f32, name="pp2", tag="pp2", bufs=3)
            tiles.append(ppt)
        nc.tensor.matmul(tiles[0][:], wm[:, WC_W2K0 + 0 : WC_W2K0 + 128], h[0][:], start=True, stop=False)
        nc.tensor.matmul(tiles[0][:], wm[:, WC_W2K1 + 0 : WC_W2K1 + 128], h[1][:], start=False, stop=True)
        nc.tensor.matmul(tiles[1][:], wm[:, WC_W2K0 + 128 : WC_W2K0 + 256], h[2][:], start=True, stop=False)
        nc.tensor.matmul(tiles[1][:], wm[:, WC_W2K1 + 128 : WC_W2K1 + 256], h[3][:], start=False, stop=True)
        nc.tensor.matmul(tiles[2][0:64, :], wm[:, WC_W2K0 + 256 : WC_W2K0 + 320], h[0][:], start=True, stop=False)
        nc.tensor.matmul(tiles[2][0:64, :], wm[:, WC_W2K1 + 256 : WC_W2K1 + 320], h[1][:], start=False, stop=True)
        nc.tensor.matmul(tiles[2][64:128, :], wm[:, WC_W2K0 + 320 : WC_W2K0 + 384], h[2][:], start=True, stop=False, tile_position=(0, 64))
        nc.tensor.matmul(tiles[2][64:128, :], wm[:, WC_W2K1 + 320 : WC_W2K1 + 384], h[3][:], start=False, stop=True, tile_position=(0, 64))
        for c in range(3):
            nb = bm[:, BC_NB2 + c : BC_NB2 + c + 1]
            if c == 2:
                # route one accumulation through ACT so DVE and PE stay balanced
                yt = dp.tile([128, NT], f16, name="y", tag="y", bufs=8)
                nc.scalar.activation(yt[:], tiles[c][:], AF.Identity)
                src = yt
            else:
                src = tiles[c]
            if o == 0:
                nc.vector.tensor_scalar(acc[c][:], src[:], nb, None, ALU.max)
            else:
                nc.vector.scalar_tensor_tensor(acc[c][:], src[:], nb, acc[c][:], op0=ALU.max, op1=ALU.add)

    def emit_rho(acc, bs):
        psz = []
        for mi in range(4):
            p = pp.tile([128, NT], f32, name="psz", tag="ph", bufs=4)
            psz.append(p)
        for mi in range(2):  # critic a
            nc.tensor.matmul(psz[mi][:], wm[:, WC_RW1A + mi * 128 : WC_RW1A + (mi + 1) * 128], acc[0][:], start=True, stop=False)
            nc.tensor.matmul(psz[mi][:], wm[:, WC_RWTA + mi * 128 : WC_RWTA + (mi + 1) * 128], acc[2][:], start=False, stop=True)
        for mi in range(2):  # critic b
            nc.tensor.matmul(psz[2 + mi][:], wm[:, WC_RW1B + mi * 128 : WC_RW1B + (mi + 1) * 128], acc[1][:], start=True, stop=False)
            nc.tensor.matmul(psz[2 + mi][:], wm[:, WC_RWTB + mi * 128 : WC_RWTB + (mi + 1) * 128], acc[2][:], start=False, stop=True)
        zr = []
        for mi in range(4):
            zt = dp.tile([128, NT], f16, name="zr", tag="zr", bufs=8)
            nc.scalar.activation(zt[:], psz[mi][:], AF.Relu, bias=bm[:, BC_RB1 + mi : BC_RB1 + mi + 1])
            zr.append(zt)
        psq = pp.tile([65, NT], f32, name="psq", tag="psq", bufs=1)
        for ci in range(2):
            ps = psq[0:1, :] if ci == 0 else psq[64:65, :]
            nc.tensor.matmul(ps, wm[:, WC_RW2 + 2 * ci : WC_RW2 + 2 * ci + 1], zr[2 * ci][:], start=True, stop=False)
            nc.tensor.matmul(ps, wm[:, WC_RW2 + 2 * ci + 1 : WC_RW2 + 2 * ci + 2], zr[2 * ci + 1][:], start=False, stop=True)
        qt = dp.tile([65, NT], f32, name="qt", tag="qt", bufs=4)
        for ci in range(2):
            pr = slice(0, 1) if ci == 0 else slice(64, 65)
            nc.scalar.activation(qt[pr, :], psq[pr, :], AF.Identity, bias=bm[pr, BC_RB2 + ci : BC_RB2 + ci + 1])
        nc.sync.dma_start(q_d[0:1, bs], qt[0:1, :])
        nc.sync.dma_start(q_d[1:2, bs], qt[64:65, :])

    pend_l2 = None
    rho_q = []
    for bt in range(BT):
        bs = slice(bt * NT, (bt + 1) * NT)
        Z, nZ = emit_body(bt)
        xos = []
        for o in range(O):
            xo = dp.tile([128, 2 * NT], f16, name="xo", tag="xo", bufs=20)
            if not no_xdma:
                nc.sync.dma_start(xo[:], xo_d[bt, o])
            else:
                nc.gpsimd.memset(xo[:], 0.25)
            xos.append(xo)
        acc = [dp.tile([128, NT], f16, name=f"ac{c}", tag="acc", bufs=6) for c in range(3)]
        for o in range(O):
            h = emit_l1(bt, o, xos[o], Z, nZ)
            if pend_l2 is not None:
                po, ph_, pacc, pbs = pend_l2
                emit_l2(po, ph_, pacc)
                if po == O - 1:
                    rho_q.append([1, (pacc, pbs)])
            pend_l2 = (o, h, acc, bs)
            for item in rho_q:
                item[0] -= 1
            if rho_q and rho_q[0][0] < 0:
                _, args = rho_q.pop(0)
                emit_rho(*args)
    po, ph_, pacc, pbs = pend_l2
    emit_l2(po, ph_, pacc)
    rho_q.append([0, (pacc, pbs)])
    for _, args in rho_q:
        emit_rho(*args)


def prep_inputs(inputs, S, core):
    f16 = np.float16
    f = np.float32
    BT = S // NT
    lo, hi = core * S, (core + 1) * S
    obs = inputs["obs"][lo:hi]
    act = inputs["act"][lo:hi]
    ctx = inputs["context_layer"][lo:hi]

    body_t = np.concatenate([obs[:, :DIM_BODY], act], axis=1).T  # [80, S]
    ctx_t = np.transpose(ctx, (1, 2, 0)).astype(f16)  # [O, 184, S]

    # xo[bt, o]: [128, 2NT]; cols 0:NT = ctx rows 0:128, cols NT:2NT =
    # ctx rows 128:184 at partitions 0:56, zeros below.
    xo = np.zeros((BT, O, 128, 2, NT), dtype=f16)
    cr = ctx_t.reshape(O, 184, BT, NT)
    xo[:, :, :, 0, :] = cr[:, 0:128].transpose(2, 0, 1, 3)
    xo[:, :, 0:56, 1, :] = cr[:, 128:184].transpose(2, 0, 1, 3)
    xo = np.ascontiguousarray(xo.reshape(BT, O, 128, 2 * NT))

    xbody = np.zeros((BT, 128, NT), dtype=f16)
    xbody[:, 0:80, :] = body_t.reshape(80, BT, NT).transpose(1, 0, 2).astype(f16)

    w1s = np.concatenate([inputs["phi_w1a"], inputs["phi_w1b"]], axis=1)  # [264, 512]
    wmega = np.zeros((128, WCOLS), dtype=f16)
    wmega[0:80, WC_BODY : WC_BODY + 512] = w1s[0:80]
    wmega[:, WC_C1 : WC_C1 + 512] = w1s[80:208]
    wmega[0:56, WC_C2 : WC_C2 + 512] = w1s[208:264]
    w2a, w2b = inputs["phi_w2a"], inputs["phi_w2b"]
    w2pack = np.concatenate([w2a[:, 0:128], w2b[:, 0:128], w2a[:, 128:192], w2b[:, 128:192]], axis=1)
    wmega[:, WC_W2K0 : WC_W2K0 + 384] = w2pack[0:128]
    wmega[:, WC_W2K1 : WC_W2K1 + 384] = w2pack[128:256]
    rw1a, rw1b = inputs["rho_w1a"], inputs["rho_w1b"]
    wmega[:, WC_RW1A : WC_RW1A + 256] = rw1a[0:128]
    wmega[:, WC_RW1B : WC_RW1B + 256] = rw1b[0:128]
    wmega[0:64, WC_RWTA : WC_RWTA + 256] = rw1a[128:192]
    wmega[64:128, WC_RWTB : WC_RWTB + 256] = rw1b[128:192]
    wmega[:, WC_RW2 + 0] = inputs["rho_w2a"][0:128, 0]
    wmega[:, WC_RW2 + 1] = inputs["rho_w2a"][128:256, 0]
    wmega[:, WC_RW2 + 2] = inputs["rho_w2b"][0:128, 0]
    wmega[:, WC_RW2 + 3] = inputs["rho_w2b"][128:256, 0]

    bmega = np.zeros((128, BCOLS), dtype=f)
    b1 = np.concatenate([inputs["phi_b1a"], inputs["phi_b1b"]])
    for m in range(4):
        bmega[:, BC_B1 + m] = b1[m * 128 : (m + 1) * 128]
        bmega[:, BC_NB1 + m] = -b1[m * 128 : (m + 1) * 128]
    b2a, b2b = inputs["phi_b2a"], inputs["phi_b2b"]
    bmega[:, BC_NB2 + 0] = -b2a[0:128]
    bmega[:, BC_NB2 + 1] = -b2b[0:128]
    bmega[:, BC_NB2 + 2] = -np.concatenate([b2a[128:192], b2b[128:192]])
    rb1a = inputs["rho_b1a"] + O * (b2a @ rw1a)
    rb1b = inputs["rho_b1b"] + O * (b2b @ rw1b)
    bmega[:, BC_RB1 + 0] = rb1a[0:128]
    bmega[:, BC_RB1 + 1] = rb1a[128:256]
    bmega[:, BC_RB1 + 2] = rb1b[0:128]
    bmega[:, BC_RB1 + 3] = rb1b[128:256]
    bmega[0, BC_RB2 + 0] = inputs["rho_b2a"][0]
    bmega[64, BC_RB2 + 1] = inputs["rho_b2b"][0]

    return {
        "xo": xo,
        "xbody": xbody,
        "wmega": np.ascontiguousarray(wmega),
        "bmega": np.ascontiguousarray(bmega),
    }


_CACHE = {}


def kernel(**inputs):
    from concourse.bass_utils import run_bass_kernel_spmd

    B = inputs["obs"].shape[0]
    assert B % N_CORES == 0
    S = B // N_CORES

    if S not in _CACHE:
        _CACHE[S] = build_nc(S)
    nc = _CACHE[S]

    in_maps = [prep_inputs(inputs, S, c) for c in range(N_CORES)]
    res = run_bass_kernel_spmd(nc, in_maps, list(range(N_CORES)))
    q1 = np.concatenate([res.results[c]["q"][0] for c in range(N_CORES)], axis=0)
    q2 = np.concatenate([res.results[c]["q"][1] for c in range(N_CORES)], axis=0)
    return (q1.reshape(-1, 1).astype(np.float32), q2.reshape(-1, 1).astype(np.float32))
